# revision 26
# baseline (speedup 1.0000x reference)
"""Gaussian KDE (brute-force, bandwidth^2 = 1) on 8 Trainium2 NeuronCores.

Math:
    out_i = log( sum_j w_j * exp(-||x_i - y_j||^2 / 2) ) - (d/2) log(2pi) - log(sum_j w_j)
          = log( sum_j exp(x_i . y_j + b_j) ) - ||x_i||^2/2 - (d/2) log(2pi) - log(sum_j w_j)
    with b_j = log(w_j) - ||y_j||^2/2.

Device work per core (queries sharded 8-way, 512 queries/core):
  - scores via K=34 bf16 matmuls (rows: y^T (32) + bias hi/lo (2)); a single
    bf16 product is accurate to ~0.02 in the exponent, well inside the 2e-2
    output tolerance.
  - The PE clock is pinned at 1.2 GHz in this environment, so matmuls are
    issued as 2-way row-tiled pairs (row groups 0 and 64): the second MM of
    each pair streams concurrently through the other half of the PE array,
    doubling throughput.  The moving operand carries different train chunks
    in partition bands 0-33 / 64-97, so each pair fills both PSUM banks of a
    [128, 1024] score tile with no data replication.
  - exp + sum per [128 queries, 1024 trains] score tile is split across
    three engines by a static pattern balancing measured per-tile costs:
      ACT tiles  (~57%): table-exp with fused accum_out, one pass over PSUM
        (~0.98us + 0.21us accumulator read per tile).
      DVE tiles  (~43%): Schraudolph int16 (bf16 bit pattern) tensor_scalar
        out of PSUM (~1.13us); then a bf16 pairwise-add tree whose first one
        or two halving levels run on the otherwise-idle GPSIMD engine
        (~1.1us/level) and whose final small reduce runs on the DVE.  The
        DVE-side reduces are emitted two tiles late so they never head the
        DVE FIFO before GPSIMD's tree output is ready.
    PSUM is 4 rotating [128,1024] tiles (8 banks): enough look-ahead that
    neither consumer ever waits on the PE refill (measured: all three
    engines ~86%% occupancy, ~205us/core vs the 295us baseline).
  - final: per-qtile reduce of the partials, ln, subtract the per-query
    constant, DMA out.
"""

import numpy as np
import ml_dtypes

_Q, _N, _D = 4096, 65536, 32
_NCORES = 8
_QSHARD = _Q // _NCORES          # 512 queries per core
_QTILES = _QSHARD // 128         # 4 psum-partition tiles per core
_SUP = 2048                      # trains per supertile (one mv DMA tile)
_NSUP = _N // _SUP               # 32 supertiles
_KROWS = 34                      # 32 y rows + bias hi/lo

_BF16 = ml_dtypes.bfloat16

# Uniform shift inside exp (folded back out via the per-query constant):
# keeps Schraudolph bit patterns positive and exp well inside range.
_SHIFT = 27.0
# Schraudolph fast-exp in bf16: exp(s) ~ bitcast_bf16(int16(C1*s + C2)).
_C1 = float(2 ** 7 / np.log(2.0))


def _c2_mean_zero():
    """127*2^7 - delta*2^7 tuned so the linear-mantissa approximation has
    zero-mean relative error over uniform mantissa fractions."""
    f = (np.arange(100000, dtype=np.float64) + 0.5) / 100000.0
    m0 = np.mean((1.0 + f) * 2.0 ** (-f))
    m1 = np.mean(2.0 ** (-f))
    delta = (m0 - 1.0) / m1
    return float(127 * 2 ** 7 - delta * 2 ** 7 + _SHIFT * _C1)


_C2 = _c2_mean_zero()

# Static engine assignment over the 256 [128,1024] score tiles, balancing
# measured per-tile costs (ACT ~1.19us; DVE tensor_scalar ~1.14us + small
# reduce; GPSIMD tree levels ~1.1us/level at half width).
_NTILES = _QTILES * _NSUP * 2    # 256 [128,1024] score tiles
_DVE_TILES = 110                 # tiles on the DVE/GPSIMD path
_D1_TILES = 48                   # of those: GPSIMD does 1 level (else 2)


def _is_dve_tile(t: int) -> bool:
    # strictly regular 3-of-7 interleave (= 109.7/256, matching the
    # balanced ACT:DVE ratio) -- no D-D adjacency, A runs of at most 2
    return t % 7 in (1, 3, 5)


def _is_d1_tile(d: int) -> bool:
    return (d * _D1_TILES) % _DVE_TILES < _D1_TILES


def _schedule():
    """Per-tile plan: 'A' = ACT exp; 'D' = DVE Schraudolph + GPSIMD tree."""
    return [('D' if _is_dve_tile(t) else 'A', False)
            for t in range(_NTILES)]


_prog_cache: dict = {}


def _build_program():
    import concourse.bass as bass  # noqa: F401
    import concourse.tile as tile
    from concourse import bacc, mybir

    f32 = mybir.dt.float32
    bf16 = mybir.dt.bfloat16
    i16 = mybir.dt.int16

    nc = bacc.Bacc("TRN2", target_bir_lowering=False, debug=False,
                   num_devices=_NCORES)

    mv_d = nc.dram_tensor("mv", [2 * _KROWS, _NSUP * 1024], bf16,
                          kind="ExternalInput")
    st_d = nc.dram_tensor("st", [128, _QSHARD], bf16, kind="ExternalInput")
    dv_d = nc.dram_tensor("dv", [128, _QTILES], f32, kind="ExternalInput")
    out_d = nc.dram_tensor("out", [128, _QTILES], f32, kind="ExternalOutput")

    with tile.TileContext(nc) as tc:
        with (
            tc.tile_pool(name="const", bufs=1) as cpool,
            tc.tile_pool(name="mv", bufs=3) as mvpool,
            tc.tile_pool(name="q16", bufs=5) as qpool,
            tc.tile_pool(name="tree", bufs=10) as tpool,
            tc.tile_pool(name="psum", bufs=4, space="PSUM") as ppool,
        ):
            st_sb = cpool.tile([128, _QSHARD], bf16)
            nc.sync.dma_start(st_sb[:], st_d[:])
            dv_sb = cpool.tile([128, _QTILES], f32)
            nc.sync.dma_start(dv_sb[:], dv_d[:])
            shift_sb = cpool.tile([128, 1], f32)
            nc.vector.memset(shift_sb[:], _SHIFT)

            # per-tile partial sums
            sall = cpool.tile([128, _NTILES], f32)
            red = cpool.tile([128, _QTILES], f32)
            fin = cpool.tile([128, _QTILES], f32)
            # warm the ACT tables early (overlapping the first DMAs); Ln
            # first so the combined natural_log_exp set can stay resident
            nc.scalar.activation(shift_sb[:], shift_sb[:],
                                 mybir.ActivationFunctionType.Ln,
                                 bias=1.0, scale=0.0)
            nc.scalar.activation(shift_sb[:], shift_sb[:],
                                 mybir.ActivationFunctionType.Exp,
                                 bias=0.0, scale=0.0)
            nc.vector.memset(shift_sb[:], _SHIFT)

            plan = _schedule()
            d_cnt = 0
            pending = []     # lagged DVE reduces: (sall col, tile, width)
            for s in range(_NSUP):
                mv_sb = mvpool.tile([128, 1024], bf16)
                nc.sync.dma_start(mv_sb[0:_KROWS, :],
                                  mv_d[0:_KROWS, s * 1024:(s + 1) * 1024])
                nc.sync.dma_start(mv_sb[64:64 + _KROWS, :],
                                  mv_d[_KROWS:2 * _KROWS,
                                       s * 1024:(s + 1) * 1024])
                for qt in range(_QTILES):
                    for h in range(2):
                        t_idx = (s * _QTILES + qt) * 2 + h
                        ps = ppool.tile([128, 1024], f32)
                        # row-tiled pair: both MMs stream concurrently
                        nc.tensor.matmul(
                            out=ps[:, 0:512],
                            lhsT=st_sb[0:_KROWS, qt * 128:(qt + 1) * 128],
                            rhs=mv_sb[0:_KROWS, h * 512:(h + 1) * 512],
                            start=True, stop=True,
                        )
                        nc.tensor.matmul(
                            out=ps[:, 512:1024],
                            lhsT=st_sb[64:64 + _KROWS,
                                       qt * 128:(qt + 1) * 128],
                            rhs=mv_sb[64:64 + _KROWS, h * 512:(h + 1) * 512],
                            start=True, stop=True,
                        )
                        col = (qt * _NSUP + s) * 2 + h
                        kind, merge = plan[t_idx]
                        if kind == 'D':
                            q16 = qpool.tile([128, 1024], i16)
                            nc.vector.tensor_scalar(
                                q16[:], ps[:], _C1, _C2,
                                mybir.AluOpType.mult, mybir.AluOpType.add)
                            qb = q16[:].bitcast(bf16)
                            t1 = tpool.tile([128, 512], bf16)
                            nc.gpsimd.tensor_tensor(
                                t1[:], qb[:, 0:512], qb[:, 512:1024],
                                mybir.AluOpType.add)
                            if _is_d1_tile(d_cnt):
                                pending.append((col, t1, 512))
                            else:
                                t2 = tpool.tile([128, 256], bf16)
                                nc.gpsimd.tensor_tensor(
                                    t2[:], t1[:, 0:256], t1[:, 256:512],
                                    mybir.AluOpType.add)
                                pending.append((col, t2, 256))
                            d_cnt += 1
                            # lag the DVE-side reduce so it never heads the
                            # DVE FIFO before GPSIMD's tree is done
                            if len(pending) > 2:
                                pcol, pt, _ = pending.pop(0)
                                nc.vector.tensor_reduce(
                                    sall[:, pcol:pcol + 1], pt[:],
                                    axis=mybir.AxisListType.X,
                                    op=mybir.AluOpType.add)
                        else:
                            nc.scalar.activation(
                                ps[:], ps[:],
                                mybir.ActivationFunctionType.Exp,
                                bias=shift_sb[:],
                                accum_out=sall[:, col:col + 1],
                            )

            for pcol, pt, _ in pending:
                nc.vector.tensor_reduce(
                    sall[:, pcol:pcol + 1], pt[:],
                    axis=mybir.AxisListType.X, op=mybir.AluOpType.add)
            pending.clear()
            for qt in range(_QTILES):
                nc.vector.tensor_reduce(
                    red[:, qt:qt + 1],
                    sall[:, qt * 2 * _NSUP:(qt + 1) * 2 * _NSUP],
                    axis=mybir.AxisListType.X, op=mybir.AluOpType.add,
                )
            nc.scalar.activation(fin[:], red[:],
                                 mybir.ActivationFunctionType.Ln)
            nc.vector.tensor_sub(fin[:], fin[:], dv_sb[:])
            nc.sync.dma_start(out_d[:], fin[:])

    nc.compile()
    return nc


def _get_program():
    if "p" not in _prog_cache:
        _prog_cache["p"] = _build_program()
    return _prog_cache["p"]


def _prep_inputs(X, X_train, sample_weight):
    X = np.ascontiguousarray(np.asarray(X, dtype=np.float32))
    Y = np.ascontiguousarray(np.asarray(X_train, dtype=np.float32))
    w = np.ascontiguousarray(np.asarray(sample_weight, dtype=np.float32))

    # per-train bias b_j = log w_j - ||y_j||^2/2, split hi/lo bf16.
    # Clip at -35 so Schraudolph bit patterns stay positive and clipped
    # terms stay utterly negligible below each row max.
    w64 = w.astype(np.float64)
    b64 = np.log(np.maximum(w64, 1e-300)) - 0.5 * np.sum(
        Y.astype(np.float64) ** 2, axis=1)
    b64 = np.clip(b64, -35.0, None)
    bhi = b64.astype(np.float32).astype(_BF16)
    blo = (b64 - bhi.astype(np.float64)).astype(np.float32).astype(_BF16)

    yT = np.ascontiguousarray(Y.T).astype(_BF16)       # [32, N]

    # moving layout: band A (SBUF partitions 0-33) gets even 512-blocks of
    # each 2048-supertile, band B (partitions 64-97) the odd blocks, so one
    # row-tiled MM pair fills both PSUM banks of a [128, 1024] score tile.
    mv = np.zeros((2 * _KROWS, _NSUP * 1024), dtype=_BF16)
    cols = np.arange(_N)
    s_idx = cols // _SUP
    j = cols % _SUP
    h = j // 1024
    r = j % 1024
    band = (r // 512)                                   # 0 = A, 1 = B
    col = s_idx * 1024 + h * 512 + (r % 512)
    rowoff = band * _KROWS
    mv[rowoff[None, :] + np.arange(32)[:, None], col[None, :]] = yT[:, cols]
    mv[rowoff + 32, col] = bhi[cols]
    mv[rowoff + 33, col] = blo[cols]

    # per-query constant: ||x||^2/2 + (d/2) log(2pi) + log(sum w) + SHIFT
    const = 0.5 * _D * np.log(2.0 * np.pi) + np.log(np.sum(w64)) + _SHIFT
    dv_all = (0.5 * np.sum(X.astype(np.float64) ** 2, axis=1)
              + const).astype(np.float32)               # [Q]

    in_maps = []
    for c in range(_NCORES):
        xq = X[c * _QSHARD:(c + 1) * _QSHARD]           # [512, 32]
        st = np.zeros((128, _QSHARD), dtype=_BF16)
        st[0:32] = xq.T.astype(_BF16)
        st[32:34] = 1.0
        st[64:96] = st[0:32]
        st[96:98] = 1.0
        dv = np.ascontiguousarray(
            dv_all[c * _QSHARD:(c + 1) * _QSHARD].reshape(_QTILES, 128).T)
        in_maps.append({"mv": mv, "st": st, "dv": dv})
    return in_maps


def _gather(results):
    out = np.empty(_Q, dtype=np.float32)
    for c in range(_NCORES):
        res = results[c]["out"]                         # [128, QTILES]
        out[c * _QSHARD:(c + 1) * _QSHARD] = res.T.reshape(_QSHARD)
    return out


def kernel(X, X_train, sample_weight, _want_timing=False):
    from concourse.bass_utils import run_bass_kernel_spmd

    nc = _get_program()
    in_maps = _prep_inputs(X, X_train, sample_weight)
    kres = run_bass_kernel_spmd(
        nc, in_maps, core_ids=list(range(_NCORES)),
        trace=bool(_want_timing),
    )
    out = _gather(kres.results)
    if _want_timing:
        return out, kres
    return out


# revision 27
# speedup vs baseline: 1.0004x; 1.0004x over previous
"""Gaussian KDE (brute-force, bandwidth^2 = 1) on 8 Trainium2 NeuronCores.

Math:
    out_i = log( sum_j w_j * exp(-||x_i - y_j||^2 / 2) ) - (d/2) log(2pi) - log(sum_j w_j)
          = log( sum_j exp(x_i . y_j + b_j) ) - ||x_i||^2/2 - (d/2) log(2pi) - log(sum_j w_j)
    with b_j = log(w_j) - ||y_j||^2/2.

Device work per core (queries sharded 8-way, 512 queries/core):
  - scores via K=34 bf16 matmuls (rows: y^T (32) + bias hi/lo (2)); a single
    bf16 product is accurate to ~0.02 in the exponent, well inside the 2e-2
    output tolerance.
  - The PE clock is pinned at 1.2 GHz in this environment, so matmuls are
    issued as 2-way row-tiled pairs (row groups 0 and 64): the second MM of
    each pair streams concurrently through the other half of the PE array,
    doubling throughput.  The moving operand carries different train chunks
    in partition bands 0-33 / 64-97, so each pair fills both PSUM banks of a
    [128, 1024] score tile with no data replication.
  - exp + sum per [128 queries, 1024 trains] score tile is split across
    three engines by a static pattern balancing measured per-tile costs:
      ACT tiles  (~57%): table-exp with fused accum_out, one pass over PSUM
        (~0.98us + 0.21us accumulator read per tile).
      DVE tiles  (~43%): Schraudolph int16 (bf16 bit pattern) tensor_scalar
        out of PSUM (~1.13us); then a bf16 pairwise-add tree whose first one
        or two halving levels run on the otherwise-idle GPSIMD engine
        (~1.1us/level) and whose final small reduce runs on the DVE.  The
        DVE-side reduces are emitted two tiles late so they never head the
        DVE FIFO before GPSIMD's tree output is ready.
    PSUM is 4 rotating [128,1024] tiles (8 banks): enough look-ahead that
    neither consumer ever waits on the PE refill (measured: all three
    engines ~86%% occupancy, ~205us/core vs the 295us baseline).
  - final: per-qtile reduce of the partials, ln, subtract the per-query
    constant, DMA out.
"""

import numpy as np
import ml_dtypes

_Q, _N, _D = 4096, 65536, 32
_NCORES = 8
_QSHARD = _Q // _NCORES          # 512 queries per core
_QTILES = _QSHARD // 128         # 4 psum-partition tiles per core
_SUP = 2048                      # trains per supertile (one mv DMA tile)
_NSUP = _N // _SUP               # 32 supertiles
_KROWS = 34                      # 32 y rows + bias hi/lo

_BF16 = ml_dtypes.bfloat16

# Uniform shift inside exp (folded back out via the per-query constant):
# keeps Schraudolph bit patterns positive and exp well inside range.
_SHIFT = 27.0
# Schraudolph fast-exp in bf16: exp(s) ~ bitcast_bf16(int16(C1*s + C2)).
_C1 = float(2 ** 7 / np.log(2.0))


def _c2_mean_zero():
    """127*2^7 - delta*2^7 tuned so the linear-mantissa approximation has
    zero-mean relative error over uniform mantissa fractions."""
    f = (np.arange(100000, dtype=np.float64) + 0.5) / 100000.0
    m0 = np.mean((1.0 + f) * 2.0 ** (-f))
    m1 = np.mean(2.0 ** (-f))
    delta = (m0 - 1.0) / m1
    return float(127 * 2 ** 7 - delta * 2 ** 7 + _SHIFT * _C1)


_C2 = _c2_mean_zero()

# Static engine assignment over the 256 [128,1024] score tiles, balancing
# measured per-tile costs (ACT ~1.19us; DVE tensor_scalar ~1.14us + small
# reduce; GPSIMD tree levels ~1.1us/level at half width).
_NTILES = _QTILES * _NSUP * 2    # 256 [128,1024] score tiles
_DVE_TILES = 110                 # tiles on the DVE/GPSIMD path
_D1_TILES = 40                   # of those: GPSIMD does 1 level (else 2);
                                 # 40/110 measured best (198.2us): shorter
                                 # GPSIMD chains beat perfectly-equal busy


def _is_dve_tile(t: int) -> bool:
    # strictly regular 3-of-7 interleave (= 109.7/256, matching the
    # balanced ACT:DVE ratio) -- no D-D adjacency, A runs of at most 2
    return t % 7 in (1, 3, 5)


def _is_d1_tile(d: int) -> bool:
    return (d * _D1_TILES) % _DVE_TILES < _D1_TILES


def _schedule():
    """Per-tile plan: 'A' = ACT exp; 'D' = DVE Schraudolph + GPSIMD tree."""
    return [('D' if _is_dve_tile(t) else 'A', False)
            for t in range(_NTILES)]


_prog_cache: dict = {}


def _build_program():
    import concourse.bass as bass  # noqa: F401
    import concourse.tile as tile
    from concourse import bacc, mybir

    f32 = mybir.dt.float32
    bf16 = mybir.dt.bfloat16
    i16 = mybir.dt.int16

    nc = bacc.Bacc("TRN2", target_bir_lowering=False, debug=False,
                   num_devices=_NCORES)

    mv_d = nc.dram_tensor("mv", [2 * _KROWS, _NSUP * 1024], bf16,
                          kind="ExternalInput")
    st_d = nc.dram_tensor("st", [128, _QSHARD], bf16, kind="ExternalInput")
    dv_d = nc.dram_tensor("dv", [128, _QTILES], f32, kind="ExternalInput")
    out_d = nc.dram_tensor("out", [128, _QTILES], f32, kind="ExternalOutput")

    with tile.TileContext(nc) as tc:
        with (
            tc.tile_pool(name="const", bufs=1) as cpool,
            tc.tile_pool(name="mv", bufs=3) as mvpool,
            tc.tile_pool(name="q16", bufs=5) as qpool,
            tc.tile_pool(name="tree", bufs=10) as tpool,
            tc.tile_pool(name="psum", bufs=4, space="PSUM") as ppool,
        ):
            st_sb = cpool.tile([128, _QSHARD], bf16)
            nc.sync.dma_start(st_sb[:], st_d[:])
            dv_sb = cpool.tile([128, _QTILES], f32)
            nc.sync.dma_start(dv_sb[:], dv_d[:])
            shift_sb = cpool.tile([128, 1], f32)
            nc.vector.memset(shift_sb[:], _SHIFT)

            # per-tile partial sums
            sall = cpool.tile([128, _NTILES], f32)
            red = cpool.tile([128, _QTILES], f32)
            fin = cpool.tile([128, _QTILES], f32)
            # warm the ACT tables early (overlapping the first DMAs); Ln
            # first so the combined natural_log_exp set can stay resident
            nc.scalar.activation(shift_sb[:], shift_sb[:],
                                 mybir.ActivationFunctionType.Ln,
                                 bias=1.0, scale=0.0)
            nc.scalar.activation(shift_sb[:], shift_sb[:],
                                 mybir.ActivationFunctionType.Exp,
                                 bias=0.0, scale=0.0)
            nc.vector.memset(shift_sb[:], _SHIFT)

            plan = _schedule()
            d_cnt = 0
            pending = []     # lagged DVE reduces: (sall col, tile, width)
            for s in range(_NSUP):
                mv_sb = mvpool.tile([128, 1024], bf16)
                nc.sync.dma_start(mv_sb[0:_KROWS, :],
                                  mv_d[0:_KROWS, s * 1024:(s + 1) * 1024])
                nc.sync.dma_start(mv_sb[64:64 + _KROWS, :],
                                  mv_d[_KROWS:2 * _KROWS,
                                       s * 1024:(s + 1) * 1024])
                for qt in range(_QTILES):
                    for h in range(2):
                        t_idx = (s * _QTILES + qt) * 2 + h
                        ps = ppool.tile([128, 1024], f32)
                        # row-tiled pair: both MMs stream concurrently
                        nc.tensor.matmul(
                            out=ps[:, 0:512],
                            lhsT=st_sb[0:_KROWS, qt * 128:(qt + 1) * 128],
                            rhs=mv_sb[0:_KROWS, h * 512:(h + 1) * 512],
                            start=True, stop=True,
                        )
                        nc.tensor.matmul(
                            out=ps[:, 512:1024],
                            lhsT=st_sb[64:64 + _KROWS,
                                       qt * 128:(qt + 1) * 128],
                            rhs=mv_sb[64:64 + _KROWS, h * 512:(h + 1) * 512],
                            start=True, stop=True,
                        )
                        col = (qt * _NSUP + s) * 2 + h
                        kind, merge = plan[t_idx]
                        if kind == 'D':
                            q16 = qpool.tile([128, 1024], i16)
                            nc.vector.tensor_scalar(
                                q16[:], ps[:], _C1, _C2,
                                mybir.AluOpType.mult, mybir.AluOpType.add)
                            qb = q16[:].bitcast(bf16)
                            t1 = tpool.tile([128, 512], bf16)
                            nc.gpsimd.tensor_tensor(
                                t1[:], qb[:, 0:512], qb[:, 512:1024],
                                mybir.AluOpType.add)
                            if _is_d1_tile(d_cnt):
                                pending.append((col, t1, 512))
                            else:
                                t2 = tpool.tile([128, 256], bf16)
                                nc.gpsimd.tensor_tensor(
                                    t2[:], t1[:, 0:256], t1[:, 256:512],
                                    mybir.AluOpType.add)
                                pending.append((col, t2, 256))
                            d_cnt += 1
                            # lag the DVE-side reduce so it never heads the
                            # DVE FIFO before GPSIMD's tree is done
                            if len(pending) > 2:
                                pcol, pt, _ = pending.pop(0)
                                nc.vector.tensor_reduce(
                                    sall[:, pcol:pcol + 1], pt[:],
                                    axis=mybir.AxisListType.X,
                                    op=mybir.AluOpType.add)
                        else:
                            nc.scalar.activation(
                                ps[:], ps[:],
                                mybir.ActivationFunctionType.Exp,
                                bias=shift_sb[:],
                                accum_out=sall[:, col:col + 1],
                            )

            for pcol, pt, _ in pending:
                nc.vector.tensor_reduce(
                    sall[:, pcol:pcol + 1], pt[:],
                    axis=mybir.AxisListType.X, op=mybir.AluOpType.add)
            pending.clear()
            for qt in range(_QTILES):
                nc.vector.tensor_reduce(
                    red[:, qt:qt + 1],
                    sall[:, qt * 2 * _NSUP:(qt + 1) * 2 * _NSUP],
                    axis=mybir.AxisListType.X, op=mybir.AluOpType.add,
                )
            nc.scalar.activation(fin[:], red[:],
                                 mybir.ActivationFunctionType.Ln)
            nc.vector.tensor_sub(fin[:], fin[:], dv_sb[:])
            nc.sync.dma_start(out_d[:], fin[:])

    nc.compile()
    return nc


def _get_program():
    if "p" not in _prog_cache:
        _prog_cache["p"] = _build_program()
    return _prog_cache["p"]


def _prep_inputs(X, X_train, sample_weight):
    X = np.ascontiguousarray(np.asarray(X, dtype=np.float32))
    Y = np.ascontiguousarray(np.asarray(X_train, dtype=np.float32))
    w = np.ascontiguousarray(np.asarray(sample_weight, dtype=np.float32))

    # per-train bias b_j = log w_j - ||y_j||^2/2, split hi/lo bf16.
    # Clip at -35 so Schraudolph bit patterns stay positive and clipped
    # terms stay utterly negligible below each row max.
    w64 = w.astype(np.float64)
    b64 = np.log(np.maximum(w64, 1e-300)) - 0.5 * np.sum(
        Y.astype(np.float64) ** 2, axis=1)
    b64 = np.clip(b64, -35.0, None)
    bhi = b64.astype(np.float32).astype(_BF16)
    blo = (b64 - bhi.astype(np.float64)).astype(np.float32).astype(_BF16)

    yT = np.ascontiguousarray(Y.T).astype(_BF16)       # [32, N]

    # moving layout: band A (SBUF partitions 0-33) gets even 512-blocks of
    # each 2048-supertile, band B (partitions 64-97) the odd blocks, so one
    # row-tiled MM pair fills both PSUM banks of a [128, 1024] score tile.
    mv = np.zeros((2 * _KROWS, _NSUP * 1024), dtype=_BF16)
    cols = np.arange(_N)
    s_idx = cols // _SUP
    j = cols % _SUP
    h = j // 1024
    r = j % 1024
    band = (r // 512)                                   # 0 = A, 1 = B
    col = s_idx * 1024 + h * 512 + (r % 512)
    rowoff = band * _KROWS
    mv[rowoff[None, :] + np.arange(32)[:, None], col[None, :]] = yT[:, cols]
    mv[rowoff + 32, col] = bhi[cols]
    mv[rowoff + 33, col] = blo[cols]

    # per-query constant: ||x||^2/2 + (d/2) log(2pi) + log(sum w) + SHIFT
    const = 0.5 * _D * np.log(2.0 * np.pi) + np.log(np.sum(w64)) + _SHIFT
    dv_all = (0.5 * np.sum(X.astype(np.float64) ** 2, axis=1)
              + const).astype(np.float32)               # [Q]

    in_maps = []
    for c in range(_NCORES):
        xq = X[c * _QSHARD:(c + 1) * _QSHARD]           # [512, 32]
        st = np.zeros((128, _QSHARD), dtype=_BF16)
        st[0:32] = xq.T.astype(_BF16)
        st[32:34] = 1.0
        st[64:96] = st[0:32]
        st[96:98] = 1.0
        dv = np.ascontiguousarray(
            dv_all[c * _QSHARD:(c + 1) * _QSHARD].reshape(_QTILES, 128).T)
        in_maps.append({"mv": mv, "st": st, "dv": dv})
    return in_maps


def _gather(results):
    out = np.empty(_Q, dtype=np.float32)
    for c in range(_NCORES):
        res = results[c]["out"]                         # [128, QTILES]
        out[c * _QSHARD:(c + 1) * _QSHARD] = res.T.reshape(_QSHARD)
    return out


def kernel(X, X_train, sample_weight, _want_timing=False):
    from concourse.bass_utils import run_bass_kernel_spmd

    nc = _get_program()
    in_maps = _prep_inputs(X, X_train, sample_weight)
    kres = run_bass_kernel_spmd(
        nc, in_maps, core_ids=list(range(_NCORES)),
        trace=bool(_want_timing),
    )
    out = _gather(kres.results)
    if _want_timing:
        return out, kres
    return out
